# Initial kernel scaffold
#
"""Trainium2 Bass kernel for a 4-layer GPT-2-style decoder (B=4, T=1024,
D=512, H=8, V=32000) with tied lm_head.

Sharding (8 cores): core c handles batch b = c//2 (body replicated across
the pair) and vocab half vh = c%2 of the lm_head. No collectives — each
core computes the full body for its batch, then logitsT[vh*16000:(vh+1)*
16000, :] (vocab-major for contiguous stores; host transposes back).

Device layout: residual stream feature-major x [128, NF, T] fp32.
Projections run as matmul(lhsT=W-tile, rhs=activation-chunk) with the
contraction on partitions. All projection matmuls are fp16 x fp16
(stationary weights get Fast Weight Load; the compiler rejects 32x16
mixes); LN gamma/beta are folded into the following weights host-side,
so device LN is just u = (x-mu)*rstd with stats from 1/D-scaled
ones-matmuls (M=128 -> broadcast) and rstd = exp(-0.5*ln(var+eps)),
keeping every layer in the natural_log_exp activation-table set (only
Gelu switches, 2 table loads/layer). Tiny dependency-pinned matmuls
inside the LN serial chain keep the PE HAM clock from re-throttling.

Attention: heads processed in row-tiled pairs (head 2j on partitions
0:64, 2j+1 on 64:128 -> concurrent K=64 matmuls), scores split by
q-halves. V is augmented with 64 ones-columns so the z-matmul emits the
softmax denominator broadcast on partitions 64:128 of the same PSUM
tile; one DVE reciprocal + one multiply normalizes per (head, q-half).

lm_head is vocab-major: whead tiles (fp16, lnf-gamma folded)
stationary, final-LN x chunks moving, output written as logitsT
[VH, T] with 4KB-contiguous rows; the host transposes and adds the
lnf-beta @ W_emb.T bias row.
"""

import os
import numpy as np
import ml_dtypes
from contextlib import ExitStack

import concourse.bass as bass
import concourse.tile as tile
from concourse import bacc, mybir
from concourse.bass_utils import run_bass_kernel_spmd

B, T, D, V, L, H = 4, 1024, 512, 32000, 4, 8
HD = D // H                 # 64
NF = D // 128               # 4 feature tiles
NTT = T // 128              # 8 token tiles
VH = V // 2                 # 16000 vocab half per core
NVG = VH // 128             # 125 vocab groups of 128
GH = 5                      # vocab groups per weight-load
LN_EPS = 1e-5
SCALE = 1.0 / np.sqrt(np.float32(D))

F32 = mybir.dt.float32
F32R = mybir.dt.float32r
BF16 = mybir.dt.bfloat16
F16 = mybir.dt.float16
AF = mybir.ActivationFunctionType
ALU = mybir.AluOpType

# The compiler rejects 32x16 matmul input mixes, so projections run
# fp16 x fp16 (11-bit mantissa, ~f32r quality): weights AND the moving
# activations (y/z/h/x-final) are fp16. The exp-probs path stays bf16
# (range), LN stats f32r x f32r, residual x fp32.
WDT = F16
WNP = np.float16

# The act-table-load pass assigns each activation the FIRST table set
# containing its function, which puts Ln in `natural_log` and Exp in
# `exp_and_others` and thrashes 5 table loads per layer. Hide Ln/Exp
# from those earlier sets so both resolve to natural_log_exp_and_others
# (set IDs keep their act_info.json positions, so the loads walrus emits
# reference the real combined set).
import functools
import concourse.hw_specs as _hw
import concourse.bacc as _bacc_mod

_orig_gat = _hw.get_activation_tables.__wrapped__


@functools.cache
def _gat(arch):
    t = {k: set(v) for k, v in _orig_gat(arch).items()}
    t["exp_and_others"].discard(AF.Exp)
    t["natural_log"].discard(AF.Ln)
    return t


_hw.get_activation_tables = _gat
for _m in (_bacc_mod,):
    if getattr(_m, "get_activation_tables", None) is not None:
        _m.get_activation_tables = _gat

_CACHE = {}


def _r(ap):
    return ap.bitcast(F32R)


def _ln(nc, pools, x_sb, y_tiles, onesd_r, onesd_b, eps_col):
    """y_f = (x - mu) * rstd, feature-major (gamma/beta are folded into
    the downstream weights host-side), processed per 512-col
    half so the serial elementwise chain of one half overlaps matmuls on
    the other. Stats via 1/D-ones matmuls (M=128 -> broadcast); rstd =
    exp(-0.5*ln(var+eps)) keeps everything in the natural_log_exp set."""
    st = pools["stat"]
    lnps = pools["ln_ps"]
    for c in range(2):
        cols = slice(c * 512, (c + 1) * 512)
        mu_ps = lnps.tile([128, 512], F32, tag="lnps", name="mu_ps")
        ex2_ps = lnps.tile([128, 512], F32, tag="lnps", name="ex2_ps")
        sq_t = pools["sq"].tile([128, NF, 512], F16, tag="sq", bufs=2,
                                name="sq_t")
        nc.scalar.square(sq_t[:], x_sb[:, :, cols])
        for f in range(NF):
            nc.tensor.matmul(mu_ps[:], _r(onesd_r[:]), _r(x_sb[:, f, cols]),
                             start=(f == 0), stop=(f == NF - 1))
        for f in range(NF):
            nc.tensor.matmul(ex2_ps[:], onesd_b[:], sq_t[:, f, :],
                             start=(f == 0), stop=(f == NF - 1))
        var_t = st.tile([128, 512], F32, tag="var", bufs=2, name="var_t")
        nc.scalar.square(var_t[:], mu_ps[:])
        nc.vector.tensor_sub(var_t[:], ex2_ps[:], var_t[:])
        nc.scalar.activation(var_t[:], var_t[:], AF.Ln, bias=eps_col[:])
        rstd = st.tile([128, 512], F32, tag="rstd", bufs=2, name="rstd")
        nc.scalar.activation(rstd[:], var_t[:], AF.Exp, scale=-0.5)
        murstd = st.tile([128, 512], F32, tag="murstd", bufs=2, name="murstd")
        nc.vector.tensor_mul(murstd[:], mu_ps[:], rstd[:])
        for f in range(NF):
            tmp = st.tile([128, 512], F32, tag="tmp", bufs=3, name="tmp")
            nc.vector.tensor_mul(tmp[:], x_sb[:, f, cols], rstd[:])
            nc.vector.tensor_sub(y_tiles[f][:, cols], tmp[:], murstd[:])


def _build():
    nc = bacc.Bacc("TRN2", target_bir_lowering=False, debug=False)

    # ---- DRAM I/O ----
    emb_tok = nc.dram_tensor("emb_tok", [128, NTT, D], F32, kind="ExternalInput").ap()
    pos_tok = nc.dram_tensor("pos_tok", [128, NTT, D], F32, kind="ExternalInput").ap()
    wqkT = nc.dram_tensor("wqkT", [L, 128, NF, 2 * D], WDT, kind="ExternalInput").ap()
    wvT = nc.dram_tensor("wvT", [L, 128, NF, D], F16, kind="ExternalInput").ap()
    woT = nc.dram_tensor("woT", [L, 128, NF, D], WDT, kind="ExternalInput").ap()
    whT = nc.dram_tensor("whT", [L, 128, NF, 4 * D], WDT, kind="ExternalInput").ap()
    wmoT = nc.dram_tensor("wmoT", [L, 128, 16, D], F16, kind="ExternalInput").ap()
    lbias = nc.dram_tensor("lbias", [L, 128, 16], F32, kind="ExternalInput").ap()
    bh_sc = nc.dram_tensor("bh_sc", [L, 128, 16], F32, kind="ExternalInput").ap()
    bv_row = nc.dram_tensor("bv_row", [L, 1, D], F32R, kind="ExternalInput").ap()
    mask2 = nc.dram_tensor("mask2", [128, 2, 128], F16, kind="ExternalInput").ap()
    ident_in = nc.dram_tensor("ident_in", [128, 128], F32, kind="ExternalInput").ap()
    ones_in = nc.dram_tensor("ones_in", [128, 128], F32R, kind="ExternalInput").ap()
    onesd_r_in = nc.dram_tensor("onesd_r_in", [128, 128], F32R, kind="ExternalInput").ap()
    onesd_b_in = nc.dram_tensor("onesd_b_in", [128, 128], F16, kind="ExternalInput").ap()
    whead = nc.dram_tensor("whead", [128, NF, NVG, 128], WDT, kind="ExternalInput").ap()
    logitsT = nc.dram_tensor("logitsT", [VH, T], F32, kind="ExternalOutput").ap()
    DEBUG = bool(int(os.environ.get("KERNEL_DEBUG", "0")))
    dbg = {}
    if DEBUG:
        for nm in ("d_x0", "d_z", "d_x1", "d_x2"):
            dbg[nm] = nc.dram_tensor(nm, [128, NF, T], F32, kind="ExternalOutput").ap()
        for nm in ("d_y1",):
            dbg[nm] = nc.dram_tensor(nm, [NF, 128, T], F16, kind="ExternalOutput").ap()
        dbg["d_qk"] = nc.dram_tensor("d_qk", [128, 2 * NF, T], F16, kind="ExternalOutput").ap()
        dbg["d_v"] = nc.dram_tensor("d_v", [128, NTT, H, 128], F16, kind="ExternalOutput").ap()
        dbg["d_xb"] = nc.dram_tensor("d_xb", [NF, 128, T], F16, kind="ExternalOutput").ap()

    with tile.TileContext(nc) as tc, ExitStack() as ctx:
        const = ctx.enter_context(tc.tile_pool(name="const", bufs=1))
        ones_sb = const.tile([128, 128], F32R)
        nc.sync.dma_start(ones_sb[:], ones_in[:])
        onesd_r = const.tile([128, 128], F32R)
        nc.sync.dma_start(onesd_r[:], onesd_r_in[:])
        onesd_b = const.tile([128, 128], F16)
        nc.sync.dma_start(onesd_b[:], onesd_b_in[:])
        ident = const.tile([128, 128], F32)
        nc.sync.dma_start(ident[:], ident_in[:])
        mask_sb = const.tile([128, 2, 128], F16)
        nc.sync.dma_start(mask_sb[:], mask2[:])
        eps_col = const.tile([128, 1], F32)
        nc.vector.memset(eps_col[:], LN_EPS)

        xp = ctx.enter_context(tc.tile_pool(name="x", bufs=1))
        x_sb = xp.tile([128, NF, T], F32)
        whp = ctx.enter_context(tc.tile_pool(name="whead", bufs=6))

        # ---- init: x = emb + pos, transpose to feature-major ----
        with tc.tile_pool(name="init", bufs=1) as initp, \
             tc.tile_pool(name="init_ps", bufs=4, space="PSUM") as initps:
            e_t = initp.tile([128, NTT, D], F32)
            p_t = initp.tile([128, NTT, D], F32)
            nc.sync.dma_start(e_t[:], emb_tok[:])
            nc.sync.dma_start(p_t[:], pos_tok[:])
            xt_t = initp.tile([128, NTT, D], F32)
            nc.vector.tensor_add(xt_t[:], e_t[:], p_t[:])
            for tt in range(NTT):
                for f in range(NF):
                    ps = initps.tile([128, 128], F32, tag="tp")
                    nc.tensor.transpose(ps[:], xt_t[:, tt, f * 128:(f + 1) * 128],
                                        ident[:])
                    nc.scalar.copy(_r(x_sb[:, f, tt * 128:(tt + 1) * 128]), ps[:])

        if DEBUG:
            nc.sync.dma_start(dbg["d_x0"][:], x_sb[:])

        # ---- layer pools ----
        lctx = ctx.enter_context(ExitStack())
        yp = lctx.enter_context(tc.tile_pool(name="y", bufs=6))
        sqp = lctx.enter_context(tc.tile_pool(name="sq", bufs=1))
        stp = lctx.enter_context(tc.tile_pool(name="stat", bufs=1))
        qkp = lctx.enter_context(tc.tile_pool(name="qk", bufs=1))
        vp = lctx.enter_context(tc.tile_pool(name="vaug", bufs=1))
        zp = lctx.enter_context(tc.tile_pool(name="z", bufs=1))
        probp = lctx.enter_context(tc.tile_pool(name="probs", bufs=4))
        rbp = lctx.enter_context(tc.tile_pool(name="rb", bufs=2))
        hp = lctx.enter_context(tc.tile_pool(name="hsb", bufs=3))
        wqk_p = lctx.enter_context(tc.tile_pool(name="wqk", bufs=1))
        wv_p = lctx.enter_context(tc.tile_pool(name="wv", bufs=1))
        wo_p = lctx.enter_context(tc.tile_pool(name="wo", bufs=1))
        wh_p = lctx.enter_context(tc.tile_pool(name="wh", bufs=1))
        wmo_p = lctx.enter_context(tc.tile_pool(name="wmo", bufs=1))
        lb_p = lctx.enter_context(tc.tile_pool(name="lbias", bufs=2))

        pools = {"sq": sqp, "stat": stp}

        for l in range(L):
            wqk_sb = wqk_p.tile([128, NF, 2 * D], WDT, tag="wqk")
            nc.sync.dma_start(wqk_sb[:], wqkT[l])
            wv_sb = wv_p.tile([128, NF, D], F16, tag="wv")
            nc.sync.dma_start(wv_sb[:], wvT[l])
            wo_sb = wo_p.tile([128, NF, D], WDT, tag="wo")
            nc.sync.dma_start(wo_sb[:], woT[l])
            wh_sb = wh_p.tile([128, NF, 4 * D], WDT, tag="wh")
            nc.sync.dma_start(wh_sb[:], whT[l])
            wmo_sb = wmo_p.tile([128, 16, D], F16, tag="wmo")
            nc.sync.dma_start(wmo_sb[:], wmoT[l])
            lb_sb = lb_p.tile([128, 16], F32, tag="lb")
            nc.sync.dma_start(lb_sb[:], lbias[l])
            bh_sb = lb_p.tile([128, 16], F32, tag="bh")
            nc.sync.dma_start(bh_sb[:], bh_sc[l])
            bv_sb = lb_p.tile([1, D], F32R, tag="bv")
            nc.sync.dma_start(bv_sb[:], bv_row[l])

            # ===== LN1 =====
            y1 = [yp.tile([128, T], F16, tag="y", name=f"y1_{f}") for f in range(NF)]
            with tc.tile_pool(name="lnps1", bufs=4, space="PSUM") as lnps:
                pools["ln_ps"] = lnps
                _ln(nc, pools, x_sb, y1, onesd_r, onesd_b, eps_col)

            if DEBUG and l == 0:
                for f in range(NF):
                    nc.sync.dma_start(dbg["d_y1"][f], y1[f][:])

            # ===== QKV =====
            qk_sb = qkp.tile([128, 2 * NF, T], F16, tag="qk")
            v_sb = vp.tile([128, NTT, H, 128], F16, tag="v")
            nc.vector.memset(v_sb[:, :, :, HD:128], 1.0)
            with tc.tile_pool(name="qkvps", bufs=5, space="PSUM") as qkvps:
                for m in range(2 * NF):
                    for c in range(2):
                        cols = slice(c * 512, (c + 1) * 512)
                        ps = qkvps.tile([128, 512], F32, tag="qkv")
                        for kt in range(NF):
                            nc.tensor.matmul(
                                ps[:], wqk_sb[:, kt, m * 128:(m + 1) * 128],
                                y1[kt][:, cols],
                                start=(kt == 0), stop=(kt == NF - 1))
                        nc.vector.tensor_scalar_add(qk_sb[:, m, cols], ps[:],
                                                    lb_sb[:, m:m + 1])
                for tt in range(NTT):
                    trng = slice(tt * 128, (tt + 1) * 128)
                    ps = qkvps.tile([128, 512], F32, tag="qkv")
                    for kt in range(NF):
                        nc.tensor.matmul(ps[:], y1[kt][:, trng],
                                         wv_sb[:, kt, :],
                                         start=(kt == 0), stop=False)
                    nc.tensor.matmul(ps[:], _r(ones_sb[0:1, :]), bv_sb[0:1, :],
                                     start=False, stop=True)
                    nc.vector.tensor_copy(
                        v_sb[:, tt, :, 0:HD],
                        ps[:].rearrange("p (h d) -> p h d", d=HD))

            if DEBUG and l == 0:
                nc.sync.dma_start(dbg["d_qk"][:], qk_sb[:])
                nc.sync.dma_start(dbg["d_v"][:], v_sb[:])

            # ===== attention (head pairs, q-halves) + Wo/residual =====
            z_sb = zp.tile([128, NF, T], F16, tag="z")
            with tc.tile_pool(name="attnps", bufs=2, space="PSUM") as sps, \
                 tc.tile_pool(name="zps", bufs=1, space="PSUM") as zps, \
                 tc.tile_pool(name="wops", bufs=2, space="PSUM") as wops:
                for qch in range(2):
                    qlo = qch * 512
                    for hpi in range(H // 2):
                        qblk, kblk = hpi, NF + hpi
                        zaug = [zps.tile([128, 512], F32, tag=f"za{hi}",
                                          name=f"za_{hi}")
                                for hi in range(2)]
                        kts = list(range(4 * (qch + 1)))
                        for kt in kts:
                            q0 = kt * 128
                            qs = max(q0, qlo)
                            n = qlo + 512 - qs
                            pt = probp.tile([128, 2, 512], F16, tag="p")
                            ps = sps.tile([128, 2, 512], F32, tag="s")
                            for hi in range(2):
                                prow = hi * 64
                                nc.tensor.matmul(
                                    ps[:, hi, 0:n],
                                    qk_sb[prow:prow + 64, kblk, q0:q0 + 128],
                                    qk_sb[prow:prow + 64, qblk, qs:qs + n],
                                    start=True, stop=True)
                            nc.scalar.activation(pt[:, :, 0:n], ps[:, :, 0:n],
                                                 AF.Exp, scale=float(SCALE))
                            if kt >= 4 * qch:
                                nc.vector.tensor_mul(pt[:, :, 0:128],
                                                     pt[:, :, 0:128], mask_sb[:])
                            for hi in range(2):
                                nc.tensor.matmul(
                                    zaug[hi][:, qs - qlo:qs - qlo + n],
                                    v_sb[:, kt, 2 * hpi + hi, :],
                                    pt[:, hi, 0:n],
                                    start=(kt == kts[0]), stop=(kt == kts[-1]))
                        for hi in range(2):
                            prow = hi * 64
                            den_sb = rbp.tile([64, 512], F32, tag="dn",
                                              name="den_sb")
                            nc.scalar.copy(den_sb[:], zaug[hi][64:128, :])
                            rden = rbp.tile([64, 512], F32, tag="rd")
                            nc.vector.reciprocal_approx_fast(
                                out=rden[:], in_=den_sb[:])
                            nc.vector.tensor_mul(
                                z_sb[prow:prow + 64, qblk, qlo:qlo + 512],
                                zaug[hi][0:64, :], rden[:])
                    # Wo projection + residual for this q-half
                    cols = slice(qlo, qlo + 512)
                    for f in range(NF):
                        ps = wops.tile([128, 512], F32, tag="wo")
                        for kt in range(NF):
                            nc.tensor.matmul(ps[:], wo_sb[:, kt, f * 128:(f + 1) * 128],
                                             z_sb[:, kt, cols],
                                             start=(kt == 0), stop=(kt == NF - 1))
                        nc.vector.scalar_tensor_tensor(
                            _r(x_sb[:, f, cols]), ps[:], lb_sb[:, 8 + f:9 + f],
                            x_sb[:, f, cols], ALU.add, ALU.add)

            # ===== LN2 =====
            y2 = [yp.tile([128, T], F16, tag="y", name=f"y2_{f}") for f in range(NF)]
            with tc.tile_pool(name="lnps2", bufs=4, space="PSUM") as lnps:
                pools["ln_ps"] = lnps
                _ln(nc, pools, x_sb, y2, onesd_r, onesd_b, eps_col)

            # ===== MLP =====
            with tc.tile_pool(name="mlpo", bufs=1, space="PSUM") as mlpo, \
                 tc.tile_pool(name="mlph", bufs=2, space="PSUM") as mlph:
                for c in range(2):
                    cols = slice(c * 512, (c + 1) * 512)
                    ops = mlpo.tile([128, NF, 512], F32, tag="mo")
                    for m in range(16):
                        hps = mlph.tile([128, 512], F32, tag="mh")
                        for kt in range(NF):
                            nc.tensor.matmul(hps[:], wh_sb[:, kt, m * 128:(m + 1) * 128],
                                             y2[kt][:, cols],
                                             start=(kt == 0), stop=(kt == NF - 1))
                        h_sb = hp.tile([128, 512], F16, tag="h")
                        nc.scalar.activation(h_sb[:], hps[:], AF.Gelu,
                                             bias=bh_sb[:, m:m + 1])
                        for f in range(NF):
                            nc.tensor.matmul(ops[:, f, :],
                                             wmo_sb[:, m, f * 128:(f + 1) * 128],
                                             h_sb[:],
                                             start=(m == 0), stop=(m == 15))
                    for f in range(NF):
                        nc.vector.scalar_tensor_tensor(
                            _r(x_sb[:, f, cols]), ops[:, f, :], lb_sb[:, 12 + f:13 + f],
                            x_sb[:, f, cols], ALU.add, ALU.add)

        if DEBUG:
            nc.sync.dma_start(dbg["d_x2"][:], x_sb[:])

        # ===== final LN (into fp16 tiles for the lm_head) =====
        xb = [xp.tile([128, T], F16, tag=f"xb{f}", name=f"xb_{f}")
              for f in range(NF)]
        with tc.tile_pool(name="lnpsf", bufs=4, space="PSUM") as lnps:
            pools["ln_ps"] = lnps
            _ln(nc, pools, x_sb, xb, onesd_r, onesd_b, eps_col)
        if DEBUG:
            for f in range(NF):
                nc.sync.dma_start(dbg["d_xb"][f], xb[f][:])
        lctx.close()

        # ===== lm_head: logitsT[v, t] = whead_tile.T @ x =====
        with tc.tile_pool(name="losb", bufs=12) as lop, \
             tc.tile_pool(name="hdps", bufs=6, space="PSUM") as hdps:
            for g in range(NVG // GH):
                wg = whp.tile([128, NF, GH, 128], WDT, tag="wg")
                nc.sync.dma_start(wg[:], whead[:, :, g * GH:(g + 1) * GH, :])
                for vgi in range(GH):
                    vg = g * GH + vgi
                    o_sb = lop.tile([128, 2, 512], F32, tag="lo")
                    for tch in range(2):
                        ps = hdps.tile([128, 512], F32, tag="hd")
                        for kt in range(NF):
                            nc.tensor.matmul(
                                ps[:], wg[:, kt, vgi, :],
                                xb[kt][:, tch * 512:(tch + 1) * 512],
                                start=(kt == 0), stop=(kt == NF - 1))
                        if tch == 0:
                            nc.scalar.copy(o_sb[:, tch, :], ps[:])
                        else:
                            nc.vector.tensor_copy(o_sb[:, tch, :], ps[:])
                    nc.sync.dma_start(logitsT[vg * 128:(vg + 1) * 128, :], o_sb[:])

    nc.compile()
    return nc


def _to_sb(wt):
    """[K, O] -> [128, K//128, O] (partition-tiled along the contraction)."""
    k, o = wt.shape
    return np.ascontiguousarray(wt.reshape(k // 128, 128, o).swapaxes(0, 1))


def _tok_tiles(x):
    """[T, D] -> [128, T//128, D] (token t=(tt*128+p) at [p, tt, :])."""
    t, d = x.shape
    return np.ascontiguousarray(x.reshape(t // 128, 128, d).swapaxes(0, 1))


def _col_sc(v):
    """[F] per-feature vector -> [128, F//128] per-partition scalar cols."""
    return np.ascontiguousarray(v.reshape(-1, 128).T)


def kernel(input_ids, W_emb, pos, Wqkv, bqkv, Wo, bo, ln1_g, ln1_b,
           ln2_g, ln2_b, Wh, bh, Wmo, bmo, lnf_g, lnf_b):
    input_ids = np.asarray(input_ids)
    W_emb = np.asarray(W_emb, dtype=np.float32)
    pos = np.asarray(pos, dtype=np.float32)
    Wqkv = np.asarray(Wqkv, dtype=np.float32)
    bqkv = np.asarray(bqkv, dtype=np.float32)
    Wo = np.asarray(Wo, dtype=np.float32)
    bo = np.asarray(bo, dtype=np.float32)
    ln1_g, ln1_b = np.asarray(ln1_g, np.float32), np.asarray(ln1_b, np.float32)
    ln2_g, ln2_b = np.asarray(ln2_g, np.float32), np.asarray(ln2_b, np.float32)
    Wh = np.asarray(Wh, dtype=np.float32)
    bh = np.asarray(bh, dtype=np.float32)
    Wmo = np.asarray(Wmo, dtype=np.float32)
    bmo = np.asarray(bmo, dtype=np.float32)
    lnf_g, lnf_b = np.asarray(lnf_g, np.float32), np.asarray(lnf_b, np.float32)

    if "nc" not in _CACHE:
        _CACHE["nc"] = _build()
    nc = _CACHE["nc"]

    # Fold LN gamma into the following weights and LN beta into their
    # biases: y = u*g + b with u = (x-mu)*rstd, so y@W.T = u@(W*g).T + W@b.
    shared = {}
    shared["pos_tok"] = _tok_tiles(pos[:T])
    shared["wqkT"] = np.stack(
        [_to_sb((Wqkv[l, 0:2 * D, :] * ln1_g[l][None, :]).T).astype(WNP)
         for l in range(L)])
    shared["wvT"] = np.stack(
        [_to_sb((Wqkv[l, 2 * D:3 * D, :] * ln1_g[l][None, :]).T).astype(np.float16)
         for l in range(L)])
    shared["woT"] = np.stack([_to_sb(Wo[l].T).astype(WNP) for l in range(L)])
    shared["whT"] = np.stack(
        [_to_sb((Wh[l] * ln2_g[l][None, :]).T).astype(WNP) for l in range(L)])
    shared["wmoT"] = np.stack([_to_sb(Wmo[l].T).astype(np.float16)
                               for l in range(L)])
    shared["lbias"] = np.stack([
        np.concatenate(
            [_col_sc(bqkv[l, 0:D] + Wqkv[l, 0:D] @ ln1_b[l]),
             _col_sc(bqkv[l, D:2 * D] + Wqkv[l, D:2 * D] @ ln1_b[l]),
             _col_sc(bo[l]), _col_sc(bmo[l])], axis=1)
        for l in range(L)])
    shared["bh_sc"] = np.stack(
        [_col_sc(bh[l] + Wh[l] @ ln2_b[l]) for l in range(L)])
    shared["bv_row"] = np.stack(
        [(bqkv[l, 2 * D:3 * D] + Wqkv[l, 2 * D:3 * D] @ ln1_b[l])[None, :]
         for l in range(L)])
    mask = np.triu(np.ones((128, 128), dtype=np.float32))
    shared["mask2"] = np.ascontiguousarray(
        np.broadcast_to(mask[:, None, :], (128, 2, 128))).astype(np.float16)
    shared["ident_in"] = np.eye(128, dtype=np.float32)
    shared["ones_in"] = np.ones((128, 128), dtype=np.float32)
    shared["onesd_r_in"] = np.full((128, 128), 1.0 / D, dtype=np.float32)
    shared["onesd_b_in"] = np.full((128, 128), 1.0 / D,
                                   dtype=np.float32).astype(np.float16)

    wembT = (W_emb * lnf_g[None, :]).T  # [D, V] with lnf gamma folded
    head_bias = W_emb @ lnf_b            # [V], added host-side at the end
    whead_halves = []
    for vh in range(2):
        half = wembT[:, vh * VH:(vh + 1) * VH]            # [D, VH]
        arr = half.reshape(NF, 128, NVG, 128).transpose(1, 0, 2, 3)
        whead_halves.append(np.ascontiguousarray(arr).astype(WNP))

    in_maps = []
    for c in range(8):
        b, vh = c // 2, c % 2
        m = dict(shared)
        m["emb_tok"] = _tok_tiles(W_emb[input_ids[b]])
        m["whead"] = whead_halves[vh]
        in_maps.append(m)

    res = run_bass_kernel_spmd(nc, in_maps, core_ids=list(range(8)),
                               trace=bool(int(os.environ.get("KERNEL_TRACE", "0"))))
    _CACHE["last_result"] = res

    out = np.empty((B, T, V), dtype=np.float32)
    for c in range(8):
        b, vh = c // 2, c % 2
        out[b, :, vh * VH:(vh + 1) * VH] = (
            res.results[c]["logitsT"].T + head_bias[None, vh * VH:(vh + 1) * VH])
    return out



# revision 14
# speedup vs baseline: 1.0629x; 1.0629x over previous
"""Trainium2 Bass kernel for a 4-layer GPT-2-style decoder (B=4, T=1024,
D=512, H=8, V=32000) with tied lm_head.

Sharding (8 cores): core c handles batch b = c//2 (body replicated across
the pair) and vocab half vh = c%2 of the lm_head. No collectives — each
core computes the full body for its batch, then logitsT[vh*16000:(vh+1)*
16000, :] (vocab-major for contiguous stores; host transposes back).

Device layout: residual stream feature-major x [128, NF, T] fp32, DMA'd
in pre-built (host does the embedding gather + pos add + transpose).
Projections run as matmul(lhsT=W-tile, rhs=activation-chunk) with the
contraction on partitions, fp16 x fp16. LN gamma/beta are folded into
the following weights host-side, so device LN is just u = (x-mu)*rstd
with stats from 1/D-scaled ones-matmuls and rstd = exp(-0.5*ln(var+eps))
(everything stays in the natural_log_exp activation-table set; only Gelu
switches tables).

The layer body is emitted as one software-pipelined stream under an
explicit 8-PSUM-bank budget: `big` (2 slots x [128,2,512] = 4 banks) for
attention score pairs / LN stat pairs / MLP out accumulators, `zps`
(1 slot x [128,2,512] = 2 banks) for the augmented-z accumulators, and
`small` (2 slots x [128,512] = 2 banks) for QKV/V/Wo outputs and MLP
hidden tiles. LN halves are interleaved into the surrounding matmul
phases (LN2(c) right after Wo(qch=c), next-layer LN1(c) right after the
MLP residual for column-half c) so the PE never starves and the HAM
clock stays at 8/8. Attention: heads processed in row-tiled pairs
(head 2j on partitions 0:64, 2j+1 on 64:128 -> concurrent K=64
matmuls), V augmented with ones-columns so the z-matmul emits the
softmax denominator in the same PSUM bank; one batched DVE reciprocal
+ two multiplies normalize per (head-pair, q-half).

lm_head is vocab-major with fp16 logits (host upcasts + adds the
lnf-beta @ W_emb.T bias row), keeping the output DMA under the PE
roofline.
"""

import os
import numpy as np
import ml_dtypes
from contextlib import ExitStack

import concourse.bass as bass
import concourse.tile as tile
from concourse import bacc, mybir
from concourse.bass_utils import run_bass_kernel_spmd

B, T, D, V, L, H = 4, 1024, 512, 32000, 4, 8
HD = D // H                 # 64
NF = D // 128               # 4 feature tiles
NTT = T // 128              # 8 token tiles
VH = V // 2                 # 16000 vocab half per core
NVG = VH // 128             # 125 vocab groups of 128
GH = 5                      # vocab groups per weight-load
LN_EPS = 1e-5
SCALE = 1.0 / np.sqrt(np.float32(D))

F32 = mybir.dt.float32
F32R = mybir.dt.float32r
BF16 = mybir.dt.bfloat16
F16 = mybir.dt.float16
AF = mybir.ActivationFunctionType
ALU = mybir.AluOpType

WDT = F16
WNP = np.float16

# The act-table-load pass assigns each activation the FIRST table set
# containing its function, which puts Ln in `natural_log` and Exp in
# `exp_and_others` and thrashes table loads per layer. Hide Ln/Exp from
# those earlier sets so both resolve to natural_log_exp_and_others.
import functools
import concourse.hw_specs as _hw
import concourse.bacc as _bacc_mod

_orig_gat = _hw.get_activation_tables.__wrapped__


@functools.cache
def _gat(arch):
    t = {k: set(v) for k, v in _orig_gat(arch).items()}
    t["exp_and_others"].discard(AF.Exp)
    t["natural_log"].discard(AF.Ln)
    return t


_hw.get_activation_tables = _gat
for _m in (_bacc_mod,):
    if getattr(_m, "get_activation_tables", None) is not None:
        _m.get_activation_tables = _gat

_CACHE = {}


def _r(ap):
    return ap.bitcast(F32R)


def _ln_half(nc, pools, x_sb, y_tiles, c, onesd_r, onesd_b, eps_col,
             ps_pool=None):
    """One 512-column half of y = (x - mu) * rstd (gamma/beta folded into
    downstream weights host-side). Stats via 1/D-ones matmuls into one
    `big` PSUM slot (mu in bank 0, E[x^2] in bank 1); squares and the
    variance arithmetic on DVE; rstd = exp(-0.5*ln(var+eps)) on ACT."""
    st = pools["st"]
    cols = slice(c * 512, (c + 1) * 512)
    sq = pools["sq"].tile([128, NF, 512], F16, tag="sq", bufs=2, name="sq")
    for f in range(NF):
        nc.vector.tensor_mul(sq[:, f, :], x_sb[:, f, cols], x_sb[:, f, cols])
    stp = (ps_pool or pools["big"]).tile([128, 2, 512], F32,
                                         tag="big" if ps_pool is None else "za",
                                         name="ln_st")
    for f in range(NF):
        nc.tensor.matmul(stp[:, 0, :], _r(onesd_r[:]), _r(x_sb[:, f, cols]),
                         start=(f == 0), stop=(f == NF - 1))
    for f in range(NF):
        nc.tensor.matmul(stp[:, 1, :], onesd_b[:], sq[:, f, :],
                         start=(f == 0), stop=(f == NF - 1))
    var = st.tile([128, 512], F32, tag="var", bufs=2, name="var")
    nc.scalar.square(var[:], stp[:, 0, :])
    nc.vector.tensor_sub(var[:], stp[:, 1, :], var[:])
    nc.scalar.activation(var[:], var[:], AF.Ln, bias=eps_col[:])
    rstd = st.tile([128, 512], F32, tag="rstd", bufs=2, name="rstd")
    nc.scalar.activation(rstd[:], var[:], AF.Exp, scale=-0.5)
    murstd = st.tile([128, 512], F32, tag="murstd", bufs=2, name="murstd")
    nc.vector.tensor_mul(murstd[:], stp[:, 0, :], rstd[:])
    for f in range(NF):
        tmp = st.tile([128, 512], F32, tag="tmp", bufs=3, name="tmp")
        nc.vector.tensor_mul(tmp[:], x_sb[:, f, cols], rstd[:])
        nc.vector.tensor_sub(y_tiles[f][:, cols], tmp[:], murstd[:])


def _build():
    nc = bacc.Bacc("TRN2", target_bir_lowering=False, debug=False)

    # ---- DRAM I/O ----
    x0_in = nc.dram_tensor("x0_in", [128, NF, T], F32R, kind="ExternalInput").ap()
    wqkT = nc.dram_tensor("wqkT", [L, 128, NF, 2 * D], WDT, kind="ExternalInput").ap()
    wvT = nc.dram_tensor("wvT", [L, 128, NF, D], F16, kind="ExternalInput").ap()
    woT = nc.dram_tensor("woT", [L, 128, NF, D], WDT, kind="ExternalInput").ap()
    whT = nc.dram_tensor("whT", [L, 128, NF, 4 * D], WDT, kind="ExternalInput").ap()
    wmoT = nc.dram_tensor("wmoT", [L, 128, 16, D], F16, kind="ExternalInput").ap()
    lbias = nc.dram_tensor("lbias", [L, 128, 16], F32, kind="ExternalInput").ap()
    bh_sc = nc.dram_tensor("bh_sc", [L, 128, 16], F32, kind="ExternalInput").ap()
    bv_row = nc.dram_tensor("bv_row", [L, 1, D], F32R, kind="ExternalInput").ap()
    mask2 = nc.dram_tensor("mask2", [128, 2, 128], F16, kind="ExternalInput").ap()
    ones_in = nc.dram_tensor("ones_in", [128, 128], F32R, kind="ExternalInput").ap()
    onesd_r_in = nc.dram_tensor("onesd_r_in", [128, 128], F32R, kind="ExternalInput").ap()
    onesd_b_in = nc.dram_tensor("onesd_b_in", [128, 128], F16, kind="ExternalInput").ap()
    whead = nc.dram_tensor("whead", [128, NF, NVG, 128], WDT, kind="ExternalInput").ap()
    logitsT = nc.dram_tensor("logitsT", [VH, T], F16, kind="ExternalOutput").ap()
    DEBUG = bool(int(os.environ.get("KERNEL_DEBUG", "0")))
    dbg = {}
    if DEBUG:
        dbg["d_y1"] = nc.dram_tensor("d_y1", [NF, 128, T], F16, kind="ExternalOutput").ap()
        dbg["d_qk"] = nc.dram_tensor("d_qk", [128, 2 * NF, T], F16, kind="ExternalOutput").ap()
        dbg["d_v"] = nc.dram_tensor("d_v", [128, NTT, H, 128], F16, kind="ExternalOutput").ap()
        dbg["d_z"] = nc.dram_tensor("d_z", [128, NF, T], F16, kind="ExternalOutput").ap()
        dbg["d_x1"] = nc.dram_tensor("d_x1", [128, NF, T], F32, kind="ExternalOutput").ap()
        dbg["d_xb"] = nc.dram_tensor("d_xb", [NF, 128, T], F16, kind="ExternalOutput").ap()

    with tile.TileContext(nc) as tc, ExitStack() as ctx:
        const = ctx.enter_context(tc.tile_pool(name="const", bufs=1))
        ones_sb = const.tile([128, 128], F32R)
        nc.sync.dma_start(ones_sb[:], ones_in[:])
        onesd_r = const.tile([128, 128], F32R)
        nc.sync.dma_start(onesd_r[:], onesd_r_in[:])
        onesd_b = const.tile([128, 128], F16)
        nc.sync.dma_start(onesd_b[:], onesd_b_in[:])
        mask_sb = const.tile([128, 2, 128], F16)
        nc.sync.dma_start(mask_sb[:], mask2[:])
        eps_col = const.tile([128, 1], F32)
        nc.vector.memset(eps_col[:], LN_EPS)
        junk = const.tile([128, 512], F32)
        nc.vector.memset(junk[:], 1.0)

        xp = ctx.enter_context(tc.tile_pool(name="x", bufs=1))
        x_sb = xp.tile([128, NF, T], F32)
        nc.sync.dma_start(_r(x_sb[:, :, 0:512]), x0_in[:, :, 0:512])
        nc.sync.dma_start(_r(x_sb[:, :, 512:1024]), x0_in[:, :, 512:1024])
        whp = ctx.enter_context(tc.tile_pool(name="whead", bufs=6))

        # ---- PSUM budget: big 2x2 + zps 1x2 + small 2x1 = 8 banks ----
        smallp = ctx.enter_context(tc.tile_pool(name="smallps", bufs=2, space="PSUM"))
        pctx = ctx.enter_context(ExitStack())
        bigp = pctx.enter_context(tc.tile_pool(name="bigps", bufs=2, space="PSUM"))
        zps = pctx.enter_context(tc.tile_pool(name="zps", bufs=1, space="PSUM"))

        # ---- SBUF pools (layer-scoped; closed before the lm_head) ----
        lctx = ctx.enter_context(ExitStack())
        yp = lctx.enter_context(tc.tile_pool(name="y", bufs=6))
        sqp = lctx.enter_context(tc.tile_pool(name="sq", bufs=1))
        stp = lctx.enter_context(tc.tile_pool(name="stat", bufs=1))
        qkp = lctx.enter_context(tc.tile_pool(name="qk", bufs=1))
        vp = lctx.enter_context(tc.tile_pool(name="vaug", bufs=1))
        zp = lctx.enter_context(tc.tile_pool(name="z", bufs=1))
        probp = lctx.enter_context(tc.tile_pool(name="probs", bufs=4))
        rbp = lctx.enter_context(tc.tile_pool(name="rb", bufs=2))
        hp = lctx.enter_context(tc.tile_pool(name="hsb", bufs=4))
        wqk_p = lctx.enter_context(tc.tile_pool(name="wqk", bufs=1))
        wv_p = lctx.enter_context(tc.tile_pool(name="wv", bufs=1))
        wo_p = lctx.enter_context(tc.tile_pool(name="wo", bufs=1))
        wh_p = lctx.enter_context(tc.tile_pool(name="wh", bufs=1))
        wmo_p = lctx.enter_context(tc.tile_pool(name="wmo", bufs=1))
        lb_p = lctx.enter_context(tc.tile_pool(name="lbias", bufs=1))

        pools = {"big": bigp, "st": stp, "sq": sqp}

        # PE warm-up: keep the HAM activity window busy while the x0 and
        # weight DMAs land, so the first real matmuls run at 8/8.
        for w in range(60):
            wps = smallp.tile([128, 512], F32, tag="sm", name="warm")
            nc.tensor.matmul(wps[:], _r(ones_sb[:]), _r(junk[:]),
                             start=True, stop=True)

        # v_sb allocated once; the ones-columns (denominator trick) are
        # memset once and persist across layers.
        v_sb = vp.tile([128, NTT, H, 128], F16, tag="v")
        nc.vector.memset(v_sb[:, :, :, 0:HD], 1.0)

        # LN1 of layer 0
        y1 = [yp.tile([128, T], F16, tag="y", name=f"y1_{f}") for f in range(NF)]
        for c in range(2):
            _ln_half(nc, pools, x_sb, y1, c, onesd_r, onesd_b, eps_col)

        if DEBUG:
            for f in range(NF):
                nc.sync.dma_start(dbg["d_y1"][f], y1[f][:])

        xb = None
        for l in range(L):
            wv_sb = wv_p.tile([128, NF, D], F16, tag="wv")
            nc.sync.dma_start(wv_sb[:], wvT[l])
            wqk_sb = wqk_p.tile([128, NF, 2 * D], WDT, tag="wqk")
            nc.sync.dma_start(wqk_sb[:], wqkT[l])
            wo_sb = wo_p.tile([128, NF, D], WDT, tag="wo")
            nc.sync.dma_start(wo_sb[:], woT[l])
            wh_sb = wh_p.tile([128, NF, 4 * D], WDT, tag="wh")
            nc.sync.dma_start(wh_sb[:], whT[l])
            wmo_sb = wmo_p.tile([128, 16, D], F16, tag="wmo")
            nc.sync.dma_start(wmo_sb[:], wmoT[l])
            lb_sb = lb_p.tile([128, 16], F32, tag="lb")
            nc.sync.dma_start(lb_sb[:], lbias[l])
            bh_sb = lb_p.tile([128, 16], F32, tag="bh")
            nc.sync.dma_start(bh_sb[:], bh_sc[l])
            bv_sb = lb_p.tile([1, D], F32R, tag="bv")
            nc.sync.dma_start(bv_sb[:], bv_row[l])

            qk_sb = qkp.tile([128, 2 * NF, T], F16, tag="qk")

            def v_proj(tt):
                trng = slice(tt * 128, (tt + 1) * 128)
                ps = smallp.tile([128, 512], F32, tag="sm", name="vps")
                for kt in range(NF):
                    nc.tensor.matmul(ps[:], y1[kt][:, trng], wv_sb[:, kt, :],
                                     start=(kt == 0), stop=False)
                nc.tensor.matmul(ps[:], _r(ones_sb[0:1, :]), bv_sb[0:1, :],
                                 start=False, stop=True)
                nc.vector.tensor_copy(
                    v_sb[:, tt, :, HD:128],
                    ps[:].rearrange("p (h d) -> p h d", d=HD))

            def qk_proj(m, c):
                cols = slice(c * 512, (c + 1) * 512)
                ps = smallp.tile([128, 512], F32, tag="sm", name="qkps")
                for kt in range(NF):
                    nc.tensor.matmul(ps[:], wqk_sb[:, kt, m * 128:(m + 1) * 128],
                                     y1[kt][:, cols],
                                     start=(kt == 0), stop=(kt == NF - 1))
                nc.vector.tensor_scalar_add(qk_sb[:, m, cols], ps[:],
                                            lb_sb[:, m:m + 1])

            # V for the first token half, then QK interleaved by head
            # pair (scores for hpi need only m=hpi and m=4+hpi), then V
            # for the second half — attention starts ~4 projections in.
            for tt in range(NTT // 2):
                v_proj(tt)
            for mp in range(NF):
                for m in (mp, NF + mp):
                    for c in range(2):
                        qk_proj(m, c)
            for tt in range(NTT // 2, NTT):
                v_proj(tt)

            # ===== attention (head pairs, q-halves) + Wo/residual =====
            y2 = [yp.tile([128, T], F16, tag="y", name=f"y2_{f}")
                  for f in range(NF)]
            z_sb = zp.tile([128, NF, T], F16, tag="z")
            for qch in range(2):
                qlo = qch * 512
                for hpi in range(H // 2):
                    zaug = zps.tile([128, 2, 512], F32, tag="za", name="zaug")
                    kts = list(range(4 * (qch + 1)))
                    for kt in kts:
                        q0 = kt * 128
                        qs = max(q0, qlo)
                        n = qlo + 512 - qs
                        sc = bigp.tile([128, 2, 512], F32, tag="big",
                                       name="sc")
                        for hi in range(2):
                            prow = hi * 64
                            nc.tensor.matmul(
                                sc[:, hi, 0:n],
                                qk_sb[prow:prow + 64, NF + hpi, q0:q0 + 128],
                                qk_sb[prow:prow + 64, hpi, qs:qs + n],
                                start=True, stop=True)
                        pt = probp.tile([128, 2, 512], F16, tag="p")
                        nc.scalar.activation(pt[:, :, 0:n], sc[:, :, 0:n],
                                             AF.Exp, scale=float(SCALE))
                        if kt >= 4 * qch:
                            nc.vector.tensor_mul(pt[:, :, 0:128],
                                                 pt[:, :, 0:128], mask_sb[:])
                        for hi in range(2):
                            nc.tensor.matmul(
                                zaug[:, hi, qs - qlo:qs - qlo + n],
                                v_sb[:, kt, 2 * hpi + hi, :],
                                pt[:, hi, 0:n],
                                start=(kt == kts[0]), stop=(kt == kts[-1]))
                    rden = rbp.tile([64, 2, 512], F32, tag="rd")
                    nc.vector.reciprocal_approx_fast(
                        out=rden[:], in_=zaug[0:64, :, :])
                    for hi in range(2):
                        prow = hi * 64
                        nc.vector.tensor_mul(
                            z_sb[prow:prow + 64, hpi, qlo:qlo + 512],
                            zaug[64:128, hi, :], rden[:, hi, :])
                # Wo projection + residual for this q-half
                cols = slice(qlo, qlo + 512)
                for f in range(NF):
                    ps = smallp.tile([128, 512], F32, tag="sm", name="wops")
                    for kt in range(NF):
                        nc.tensor.matmul(ps[:],
                                         wo_sb[:, kt, f * 128:(f + 1) * 128],
                                         z_sb[:, kt, cols],
                                         start=(kt == 0), stop=(kt == NF - 1))
                    nc.vector.scalar_tensor_tensor(
                        _r(x_sb[:, f, cols]), ps[:], lb_sb[:, 8 + f:9 + f],
                        x_sb[:, f, cols], ALU.add, ALU.add)
                # LN2 for this column half overlaps the other q-half
                _ln_half(nc, pools, x_sb, y2, qch, onesd_r, onesd_b, eps_col)

            if DEBUG and l == 0:
                nc.sync.dma_start(dbg["d_qk"][:], qk_sb[:])
                nc.sync.dma_start(dbg["d_v"][:], v_sb[:])
                nc.sync.dma_start(dbg["d_z"][:], z_sb[:])
                nc.sync.dma_start(dbg["d_x1"][:], x_sb[:])

            # ===== MLP (out accumulates into the two big slots) =====
            if l == L - 1:
                ynext = [xp.tile([128, T], F16, tag=f"xb{f}", name=f"xb_{f}")
                         for f in range(NF)]
                xb = ynext
            else:
                ynext = [yp.tile([128, T], F16, tag="y", name=f"y1n_{f}")
                         for f in range(NF)]
            for c in range(2):
                cols = slice(c * 512, (c + 1) * 512)
                ops2 = [bigp.tile([128, 2, 512], F32, tag="big",
                                  name=f"mo{i}") for i in range(2)]
                for m in range(16):
                    hps = smallp.tile([128, 512], F32, tag="sm", name="hps")
                    for kt in range(NF):
                        nc.tensor.matmul(hps[:],
                                         wh_sb[:, kt, m * 128:(m + 1) * 128],
                                         y2[kt][:, cols],
                                         start=(kt == 0), stop=(kt == NF - 1))
                    h_sb = hp.tile([128, 512], F16, tag="h")
                    nc.scalar.activation(h_sb[:], hps[:], AF.Gelu,
                                         bias=bh_sb[:, m:m + 1])
                    for f in range(NF):
                        nc.tensor.matmul(ops2[f // 2][:, f % 2, :],
                                         wmo_sb[:, m, f * 128:(f + 1) * 128],
                                         h_sb[:],
                                         start=(m == 0), stop=(m == 15))
                for f in range(NF):
                    nc.vector.scalar_tensor_tensor(
                        _r(x_sb[:, f, cols]), ops2[f // 2][:, f % 2, :],
                        lb_sb[:, 12 + f:13 + f],
                        x_sb[:, f, cols], ALU.add, ALU.add)
                # next-layer LN1 (or final LN) for this half overlaps the
                # other MLP half; stats use the idle zaug PSUM slot.
                _ln_half(nc, pools, x_sb, ynext, c, onesd_r, onesd_b, eps_col,
                         ps_pool=zps)
            y1 = ynext

        if DEBUG:
            for f in range(NF):
                nc.sync.dma_start(dbg["d_xb"][f], xb[f][:])
        lctx.close()
        pctx.close()

        # ===== lm_head: logitsT[v, t] = whead_tile.T @ x, fp16 out =====
        with tc.tile_pool(name="losb", bufs=10) as lop, \
             tc.tile_pool(name="headps", bufs=3, space="PSUM") as headp:
            for g in range(NVG // GH):
                wg = whp.tile([128, NF, GH, 128], WDT, tag="wg")
                nc.sync.dma_start(wg[:], whead[:, :, g * GH:(g + 1) * GH, :])
                for vgi in range(GH):
                    vg = g * GH + vgi
                    o_sb = lop.tile([128, 2, 512], F16, tag="lo")
                    ps = headp.tile([128, 2, 512], F32, tag="hd", name="hdps")
                    for tch in range(2):
                        for kt in range(NF):
                            nc.tensor.matmul(
                                ps[:, tch, :], wg[:, kt, vgi, :],
                                xb[kt][:, tch * 512:(tch + 1) * 512],
                                start=(kt == 0), stop=(kt == NF - 1))
                    nc.scalar.copy(o_sb[:, 0, :], ps[:, 0, :])
                    nc.vector.tensor_copy(o_sb[:, 1, :], ps[:, 1, :])
                    nc.sync.dma_start(logitsT[vg * 128:(vg + 1) * 128, :],
                                      o_sb[:])

    nc.compile()
    return nc


def _to_sb(wt):
    """[K, O] -> [128, K//128, O] (partition-tiled along the contraction)."""
    k, o = wt.shape
    return np.ascontiguousarray(wt.reshape(k // 128, 128, o).swapaxes(0, 1))


def _feat_tiles(x):
    """[T, D] -> [128, D//128, T] feature-major (feature f*128+p at [p, f, t])."""
    t, d = x.shape
    return np.ascontiguousarray(x.T.reshape(d // 128, 128, t).swapaxes(0, 1))


def _col_sc(v):
    """[F] per-feature vector -> [128, F//128] per-partition scalar cols."""
    return np.ascontiguousarray(v.reshape(-1, 128).T)


def kernel(input_ids, W_emb, pos, Wqkv, bqkv, Wo, bo, ln1_g, ln1_b,
           ln2_g, ln2_b, Wh, bh, Wmo, bmo, lnf_g, lnf_b):
    input_ids = np.asarray(input_ids)
    W_emb = np.asarray(W_emb, dtype=np.float32)
    pos = np.asarray(pos, dtype=np.float32)
    Wqkv = np.asarray(Wqkv, dtype=np.float32)
    bqkv = np.asarray(bqkv, dtype=np.float32)
    Wo = np.asarray(Wo, dtype=np.float32)
    bo = np.asarray(bo, dtype=np.float32)
    ln1_g, ln1_b = np.asarray(ln1_g, np.float32), np.asarray(ln1_b, np.float32)
    ln2_g, ln2_b = np.asarray(ln2_g, np.float32), np.asarray(ln2_b, np.float32)
    Wh = np.asarray(Wh, dtype=np.float32)
    bh = np.asarray(bh, dtype=np.float32)
    Wmo = np.asarray(Wmo, dtype=np.float32)
    bmo = np.asarray(bmo, dtype=np.float32)
    lnf_g, lnf_b = np.asarray(lnf_g, np.float32), np.asarray(lnf_b, np.float32)

    if "nc" not in _CACHE:
        _CACHE["nc"] = _build()
    nc = _CACHE["nc"]

    # Fold LN gamma into the following weights and LN beta into their
    # biases: y = u*g + b with u = (x-mu)*rstd, so y@W.T = u@(W*g).T + W@b.
    shared = {}
    shared["wqkT"] = np.stack(
        [_to_sb((Wqkv[l, 0:2 * D, :] * ln1_g[l][None, :]).T).astype(WNP)
         for l in range(L)])
    shared["wvT"] = np.stack(
        [_to_sb((Wqkv[l, 2 * D:3 * D, :] * ln1_g[l][None, :]).T).astype(np.float16)
         for l in range(L)])
    shared["woT"] = np.stack([_to_sb(Wo[l].T).astype(WNP) for l in range(L)])
    shared["whT"] = np.stack(
        [_to_sb((Wh[l] * ln2_g[l][None, :]).T).astype(WNP) for l in range(L)])
    shared["wmoT"] = np.stack([_to_sb(Wmo[l].T).astype(np.float16)
                               for l in range(L)])
    shared["lbias"] = np.stack([
        np.concatenate(
            [_col_sc(bqkv[l, 0:D] + Wqkv[l, 0:D] @ ln1_b[l]),
             _col_sc(bqkv[l, D:2 * D] + Wqkv[l, D:2 * D] @ ln1_b[l]),
             _col_sc(bo[l]), _col_sc(bmo[l])], axis=1)
        for l in range(L)])
    shared["bh_sc"] = np.stack(
        [_col_sc(bh[l] + Wh[l] @ ln2_b[l]) for l in range(L)])
    shared["bv_row"] = np.stack(
        [(bqkv[l, 2 * D:3 * D] + Wqkv[l, 2 * D:3 * D] @ ln1_b[l])[None, :]
         for l in range(L)])
    mask = np.triu(np.ones((128, 128), dtype=np.float32))
    shared["mask2"] = np.ascontiguousarray(
        np.broadcast_to(mask[:, None, :], (128, 2, 128))).astype(np.float16)
    shared["ones_in"] = np.ones((128, 128), dtype=np.float32)
    shared["onesd_r_in"] = np.full((128, 128), 1.0 / D, dtype=np.float32)
    shared["onesd_b_in"] = np.full((128, 128), 1.0 / D,
                                   dtype=np.float32).astype(np.float16)

    wembT = (W_emb * lnf_g[None, :]).T  # [D, V] with lnf gamma folded
    head_bias = W_emb @ lnf_b            # [V], added host-side at the end
    whead_halves = []
    for vh in range(2):
        half = wembT[:, vh * VH:(vh + 1) * VH]            # [D, VH]
        arr = half.reshape(NF, 128, NVG, 128).transpose(1, 0, 2, 3)
        whead_halves.append(np.ascontiguousarray(arr).astype(WNP))

    in_maps = []
    for c in range(8):
        b, vh = c // 2, c % 2
        m = dict(shared)
        m["x0_in"] = _feat_tiles(W_emb[input_ids[b]] + pos[:T])
        m["whead"] = whead_halves[vh]
        in_maps.append(m)

    res = run_bass_kernel_spmd(nc, in_maps, core_ids=list(range(8)),
                               trace=bool(int(os.environ.get("KERNEL_TRACE", "0"))))
    _CACHE["last_result"] = res

    out = np.empty((B, T, V), dtype=np.float32)
    for c in range(8):
        b, vh = c // 2, c % 2
        out[b, :, vh * VH:(vh + 1) * VH] = (
            res.results[c]["logitsT"].T.astype(np.float32)
            + head_bias[None, vh * VH:(vh + 1) * VH])
    return out
